# revision 1
# baseline (speedup 1.0000x reference)
"""Causal self-attention (GQA + RoPE + QK-RMSNorm) on 8 trn2 NeuronCores.

Reference (B=2, T=2048, C=2048, 16 q-heads / 4 kv-heads, head_dim 128):
    q = rms_norm(rope(x @ Wq)) / sqrt(128); k = rms_norm(rope(x @ Wk))
    att = softmax_causal(q k^T / sqrt(128)); y = (att @ v) @ Wp

Sharding: core = 4*b + g  (b = batch 0..1, g = head-group 0..3).
Each core computes q-heads 4g..4g+3 (all mapping to kv-head g under GQA),
attends over the full causal sequence of its batch, AllGathers attention
outputs within its batch group, and computes a 512-column slice of the
output projection. Host concatenates.

On-chip layout is transposed activations: X^T, Q^T, K^T, S^T, P^T, Y^T all
[feature, t]; every matmul contracts on the partition axis. Softmax runs
without max-subtraction: q,k are rms-normalized so |q.k|/128 <= 1.

Performance structure (vs the naive version):
 - elementwise rope/normalize runs in bf16 SBUF on DVE (2-4x DVE modes);
   PSUM evacuation via the scalar engine (activation Copy).
 - per-key rms scale is folded into K^T, so softmax exp needs no per-block
   scale and runs over PAIRS of S blocks ([128,1024] activations).
 - output projection accumulates all contributions in PSUM (no vector adds);
   heads 0-2 are projected right after P2 (hiding AllGather latency), head 3
   joins via one spill+add per output tile.
 - weight/trig loads are single batched DMAs spread across engine queues.
"""

import ml_dtypes
import numpy as np

B, T, C = 2, 2048, 2048
NH, NKV, HD = 16, 4, 128
G = 4  # q-heads per core
EPS = 1e-6
NCB = C // 128  # 16 contraction blocks
NTCH = T // 512  # 4 t-chunks
NTKB = T // 128  # 16 key blocks

_CACHE = {}


def _build():
    import concourse.mybir as mybir
    import concourse.tile as tile
    from concourse import bacc
    from concourse.masks import make_identity
    from contextlib import ExitStack

    F32 = mybir.dt.float32
    BF16 = mybir.dt.bfloat16
    AF = mybir.ActivationFunctionType

    nc = bacc.Bacc(None, target_bir_lowering=False, num_devices=8)

    xT = nc.dram_tensor("xT", [C, T], BF16, kind="ExternalInput")
    wq = nc.dram_tensor("wq", [C, G * HD], BF16, kind="ExternalInput")
    wk = nc.dram_tensor("wk", [C, HD], BF16, kind="ExternalInput")
    wv = nc.dram_tensor("wv", [C, HD], BF16, kind="ExternalInput")
    wp = nc.dram_tensor("wp", [C, G * HD], BF16, kind="ExternalInput")
    cosT = nc.dram_tensor("cosT", [128, T], BF16, kind="ExternalInput")
    sinT = nc.dram_tensor("sinT", [128, T], BF16, kind="ExternalInput")
    masks = nc.dram_tensor("masks", [4, 128, 512], BF16, kind="ExternalInput")
    outT = nc.dram_tensor("outT", [G * HD, T], BF16, kind="ExternalOutput")

    with tile.TileContext(nc) as tc:
        with ExitStack() as S:
            dram = S.enter_context(tc.tile_pool(name="dram", bufs=1, space="DRAM"))
            ag_in = dram.tile([G * HD, T], BF16)
            ag_outs = [
                dram.tile([4 * HD, T], BF16, name=f"ag_out_{q}") for q in range(3)
            ]
            ag_out3h = [
                dram.tile([4 * HD, T // 2], BF16, name=f"ag_out_3{i}")
                for i in range(2)
            ]
            ag_in3h = [
                dram.tile([HD, T // 2], BF16, name=f"ag_in_3{i}") for i in range(2)
            ]

            consts = S.enter_context(tc.tile_pool(name="consts", bufs=1))
            ones_bf = consts.tile([128, 1], BF16)
            nc.vector.memset(ones_bf[:], 1.0)
            eps_k = consts.tile([1, 1], F32)
            nc.vector.memset(eps_k[:], EPS)
            eps_q = consts.tile([1, 1], F32)
            nc.vector.memset(eps_q[:], float(HD * HD) * EPS)
            ident_bf = consts.tile([128, 128], BF16)
            make_identity(nc, ident_bf[:])

            wpool = S.enter_context(tc.tile_pool(name="w", bufs=1))
            wq_sb = wpool.tile([128, NCB, G * HD], BF16)
            wk_sb = wpool.tile([128, NCB, HD], BF16)
            wv_sb = wpool.tile([128, NCB, HD], BF16)
            wp_sb = wpool.tile([128, NCB, G * HD], BF16)
            # fine-grained weight loads: separate dma_starts run on separate
            # DMA engines concurrently (one big batched DMA serializes on a
            # single HW queue and takes 4-8x longer wall-clock).
            wkr = wk.rearrange("(cb p) n -> p cb n", p=128)
            wvr = wv.rearrange("(cb p) n -> p cb n", p=128)
            wqr = wq.rearrange("(cb p) n -> p cb n", p=128)
            wpr = wp.rearrange("(cb p) n -> p cb n", p=128)
            for i in range(2):
                cbs = slice(8 * i, 8 * i + 8)
                nc.scalar.dma_start(out=wk_sb[:, cbs, :], in_=wkr[:, cbs, :])

            trig = S.enter_context(tc.tile_pool(name="trig", bufs=1))
            cos_sb = trig.tile([128, T], BF16)
            sin_sb = trig.tile([128, T], BF16)
            masks_sb = trig.tile([128, 4, 512], BF16)

            acts = S.enter_context(tc.tile_pool(name="acts", bufs=1))
            qT_sb = acts.tile([128, G, T], BF16)
            kT_sb = acts.tile([128, T], BF16)
            v_sb = acts.tile([128, NTKB, HD], BF16)

            xt_pool = S.enter_context(tc.tile_pool(name="xt", bufs=8))
            tmp = S.enter_context(tc.tile_pool(name="tmp", bufs=2))
            rowp = S.enter_context(tc.tile_pool(name="rowp", bufs=4))
            pt_pool = S.enter_context(tc.tile_pool(name="pt", bufs=3))

            pP1 = ExitStack()
            pp = pP1.enter_context(tc.tile_pool(name="pp", bufs=4, space="PSUM"))
            ptr = pP1.enter_context(tc.tile_pool(name="ptr", bufs=2, space="PSUM"))
            psm = pP1.enter_context(tc.tile_pool(name="psm", bufs=2, space="PSUM"))

            def rope_norm(dst, psrc, tcs, sqrt_scale, sqrt_bias):
                """dst = rope(psrc) / sqrt(sqrt_scale*ssq + bias), bf16 math."""
                xb = tmp.tile([128, 512], BF16, tag="xb")
                nc.scalar.copy(out=xb[:], in_=psrc)
                rot = tmp.tile([128, 512], BF16, tag="rot")
                # sin_sb rows 0-63 hold +sin, rows 64-127 hold -sin, so each
                # tensor_tensor reads both SBUF operands at the same base
                # partition (compiler constraint NCC_IBIR297).
                nc.vector.tensor_mul(rot[0:64, :], xb[64:128, :], sin_sb[64:128, tcs])
                nc.vector.tensor_mul(rot[64:128, :], xb[0:64, :], sin_sb[0:64, tcs])
                xc = tmp.tile([128, 512], BF16, tag="xc")
                nc.vector.tensor_mul(xc[:], xb[:], cos_sb[:, tcs])
                ro = tmp.tile([128, 512], BF16, tag="ro")
                nc.vector.tensor_add(ro[:], xc[:], rot[:])
                sq = tmp.tile([128, 512], BF16, tag="sq")
                nc.vector.tensor_mul(sq[:], ro[:], ro[:])
                ps_ss = psm.tile([1, 512], F32, tag="psm")
                nc.tensor.matmul(ps_ss[:], ones_bf[:], sq[:], start=True, stop=True)
                srow = rowp.tile([1, 512], F32, tag="srow")
                nc.scalar.activation(
                    out=srow[:], in_=ps_ss[:], func=AF.Sqrt,
                    scale=sqrt_scale, bias=sqrt_bias,
                )
                rrow = rowp.tile([1, 512], F32, tag="rrow")
                nc.vector.reciprocal_approx_fast(out=rrow[:], in_=srow[:])
                bc = tmp.tile([128, 512], F32, tag="bc")
                nc.gpsimd.partition_broadcast(bc[:], rrow[:])
                nc.vector.tensor_mul(dst, ro[:], bc[:])

            # ---- phase 1: Q/K/V projections + RoPE + RMS-norm ----
            for tch in range(NTCH):
                tcs = slice(512 * tch, 512 * tch + 512)
                xts = []
                if tch == 0:
                    # first chunk gates the whole pipeline: 16 small DMAs
                    # round-robined over all three queues land ~3x sooner
                    # than 8 coarse ones on two queues.
                    engs = [nc.sync, nc.gpsimd, nc.scalar]
                    for gx in range(4):
                        xt = xt_pool.tile([128, 4, 512], BF16, tag="xt")
                        xr = xT[512 * gx : 512 * (gx + 1), tcs].rearrange(
                            "(cb p) t -> p cb t", p=128
                        )
                        for j in range(4):
                            engs[(4 * gx + j) % 3].dma_start(
                                out=xt[:, j : j + 1, :], in_=xr[:, j : j + 1, :]
                            )
                        xts.append(xt)
                    # remaining weights/tables issue behind chunk-0 data
                    nc.scalar.dma_start(out=cos_sb[:], in_=cosT[:])
                    nc.scalar.dma_start(out=sin_sb[:], in_=sinT[:])
                    for i in range(2):
                        cbs = slice(8 * i, 8 * i + 8)
                        nc.scalar.dma_start(out=wv_sb[:, cbs, :], in_=wvr[:, cbs, :])
                    for i in range(4):
                        cbs = slice(4 * i, 4 * i + 4)
                        eng = nc.sync if i % 2 == 0 else nc.scalar
                        eng.dma_start(out=wq_sb[:, cbs, :], in_=wqr[:, cbs, :])
                else:
                    for gx in range(4):
                        xt = xt_pool.tile([128, 4, 512], BF16, tag="xt")
                        xr = xT[512 * gx : 512 * (gx + 1), tcs].rearrange(
                            "(cb p) t -> p cb t", p=128
                        )
                        eng = nc.sync if gx < 2 else nc.gpsimd
                        eng.dma_start(out=xt[:, 0:2, :], in_=xr[:, 0:2, :])
                        eng.dma_start(out=xt[:, 2:4, :], in_=xr[:, 2:4, :])
                        xts.append(xt)

                def xmov(cb):
                    return xts[cb // 4][:, cb % 4, :]

                # K^T chunk, rms-scale folded in
                ps_k = pp.tile([128, 512], F32, tag="proj")
                for cb in range(NCB):
                    nc.tensor.matmul(
                        ps_k[:], wk_sb[:, cb, :], xmov(cb),
                        start=(cb == 0), stop=(cb == NCB - 1),
                    )
                rope_norm(kT_sb[:, tcs], ps_k[:], tcs, 1.0 / HD, eps_k[:])

                # V^T chunk, PE-transposed into v_sb
                ps_v = pp.tile([128, 512], F32, tag="proj")
                for cb in range(NCB):
                    nc.tensor.matmul(
                        ps_v[:], wv_sb[:, cb, :], xmov(cb),
                        start=(cb == 0), stop=(cb == NCB - 1),
                    )
                vb = tmp.tile([128, 512], BF16, tag="vb")
                nc.scalar.copy(out=vb[:], in_=ps_v[:])
                for tt in range(4):
                    ps_tr = ptr.tile([128, 128], BF16, tag="tr")
                    nc.tensor.transpose(
                        ps_tr[:], vb[:, 128 * tt : 128 * (tt + 1)], ident_bf[:]
                    )
                    nc.vector.tensor_copy(out=v_sb[:, 4 * tch + tt, :], in_=ps_tr[:])

                # Q^T per head (rq_eff = 1/sqrt(HD*ssq + HD^2*eps) folds the
                # double 1/sqrt(HD) attention scaling)
                for hq in range(G):
                    ps_q = pp.tile([128, 512], F32, tag="proj")
                    for cb in range(NCB):
                        nc.tensor.matmul(
                            ps_q[:],
                            wq_sb[:, cb, 128 * hq : 128 * (hq + 1)],
                            xmov(cb),
                            start=(cb == 0), stop=(cb == NCB - 1),
                        )
                    rope_norm(qT_sb[:, hq, tcs], ps_q[:], tcs, float(HD), eps_q[:])

            # masks are first needed in phase 2, wp only in phase 4: issue
            # them after the startup-critical chunk loads have drained.
            nc.gpsimd.dma_start(
                out=masks_sb[:], in_=masks.rearrange("d p m -> p d m")
            )
            for i in range(4):
                cbs = slice(4 * i, 4 * i + 4)
                nc.gpsimd.dma_start(out=wp_sb[:, cbs, :], in_=wpr[:, cbs, :])

            pP1.close()

            # ---- phase 2: causal attention, S^T/P^T orientation ----
            psp = S.enter_context(tc.tile_pool(name="psp", bufs=2, space="PSUM"))
            psy = S.enter_context(tc.tile_pool(name="psy", bufs=2, space="PSUM"))
            prs = S.enter_context(tc.tile_pool(name="prs", bufs=2, space="PSUM"))

            for hq in range(G):
                for tqc in range(NTCH):
                    tqs = slice(512 * tqc, 512 * tqc + 512)
                    nblk = 4 * tqc + 4
                    ps_y = psy.tile([128, 512], F32, tag="psy")
                    rs = prs.tile([1, 512], F32, tag="rs")
                    for pr in range(nblk // 2):
                        sp = psp.tile([128, 1024], F32, tag="spair")
                        for h2 in range(2):
                            tkb = 2 * pr + h2
                            nc.tensor.matmul(
                                sp[:, 512 * h2 : 512 * (h2 + 1)],
                                kT_sb[:, 128 * tkb : 128 * (tkb + 1)],
                                qT_sb[:, hq, tqs],
                                start=True, stop=True,
                            )
                        pT = pt_pool.tile([128, 1024], BF16, tag="pt")
                        nc.scalar.activation(out=pT[:], in_=sp[:], func=AF.Exp)
                        for h2 in range(2):
                            tkb = 2 * pr + h2
                            hs = slice(512 * h2, 512 * (h2 + 1))
                            d = tkb - 4 * tqc
                            if d >= 0:
                                nc.vector.tensor_mul(
                                    pT[:, hs], pT[:, hs], masks_sb[:, d, :]
                                )
                            nc.tensor.matmul(
                                rs[:], ones_bf[:], pT[:, hs],
                                start=(tkb == 0), stop=(tkb == nblk - 1),
                            )
                            nc.tensor.matmul(
                                ps_y[:], v_sb[:, tkb, :], pT[:, hs],
                                start=(tkb == 0), stop=(tkb == nblk - 1),
                            )
                    rrow = rowp.tile([1, 512], F32, tag="rrow2")
                    nc.vector.reciprocal_approx_fast(out=rrow[:], in_=rs[:])
                    bc = tmp.tile([128, 512], F32, tag="bc2")
                    nc.gpsimd.partition_broadcast(bc[:], rrow[:])
                    yT = tmp.tile([128, 512], BF16, tag="yT")
                    nc.vector.tensor_mul(yT[:], ps_y[:], bc[:])
                    if hq < 3:
                        nc.sync.dma_start(
                            out=ag_in[128 * hq : 128 * (hq + 1), tqs], in_=yT[:]
                        )
                    else:
                        nc.sync.dma_start(
                            out=ag_in3h[tqc // 2][
                                :, 512 * (tqc % 2) : 512 * (tqc % 2) + 512
                            ],
                            in_=yT[:],
                        )
                    if hq == 3 and tqc % 2 == 1:
                        # half-sequence AllGathers for the last head: the
                        # first half lands before phase 2 ends, shrinking the
                        # part-B tail wait to the second (small) collective.
                        half = tqc // 2
                        nc.gpsimd.collective_compute(
                            "AllGather",
                            mybir.AluOpType.bypass,
                            replica_groups=[[0, 1, 2, 3], [4, 5, 6, 7]],
                            ins=[ag_in3h[half][:]],
                            outs=[ag_out3h[half][:]],
                        )
                if hq < 3:
                    nc.gpsimd.collective_compute(
                        "AllGather",
                        mybir.AluOpType.bypass,
                        replica_groups=[[0, 1, 2, 3], [4, 5, 6, 7]],
                        ins=[ag_in[HD * hq : HD * (hq + 1), :]],
                        outs=[ag_outs[hq][:]],
                    )

            # ---- phase 4: output projection, PSUM-accumulated ----
            # part A: heads 0-2 (their AllGathers land during phase 2);
            # part B: head 3 joins via one spill+add per output tile.
            from concourse.tile_rust import add_dep_helper

            ytA_pool = S.enter_context(tc.tile_pool(name="ytA", bufs=16))
            ytB_pool = S.enter_context(tc.tile_pool(name="ytB", bufs=8))
            spill_pool = S.enter_context(tc.tile_pool(name="spill", bufs=16))
            osb_pool = S.enter_context(tc.tile_pool(name="osb", bufs=3))
            spills = {}

            for tch in range(NTCH):
                tcs = slice(512 * tch, 512 * tch + 512)
                ytsA = []
                for q in range(3):
                    row = []
                    for r in range(4):
                        yt = ytA_pool.tile([128, 512], BF16, tag="ytA")
                        nc.sync.dma_start(
                            out=yt[:], in_=ag_outs[q][128 * r : 128 * (r + 1), tcs]
                        )
                        row.append(yt)
                    ytsA.append(row)
                for cob in range(4):
                    ps_o = psy.tile([128, 512], F32, tag="psy")
                    for q in range(3):
                        for r in range(4):
                            nc.tensor.matmul(
                                ps_o[:],
                                wp_sb[:, 4 * r + q, 128 * cob : 128 * (cob + 1)],
                                ytsA[q][r][:],
                                start=(q == 0 and r == 0),
                                stop=(q == 2 and r == 3),
                            )
                    sp_t = spill_pool.tile(
                        [128, 512], BF16, tag="spill", name=f"spill{tch}_{cob}"
                    )
                    nc.scalar.copy(out=sp_t[:], in_=ps_o[:])
                    spills[(tch, cob)] = sp_t

            for tch in range(NTCH):
                tcs = slice(512 * tch, 512 * tch + 512)
                ytB = []
                for r in range(4):
                    yt = ytB_pool.tile([128, 512], BF16, tag="ytB")
                    nc.sync.dma_start(
                        out=yt[:],
                        in_=ag_out3h[tch // 2][
                            128 * r : 128 * (r + 1),
                            512 * (tch % 2) : 512 * (tch % 2) + 512,
                        ],
                    )
                    ytB.append(yt)
                for cob in range(4):
                    ps_o = psy.tile([128, 512], F32, tag="psy")
                    for r in range(4):
                        nc.tensor.matmul(
                            ps_o[:],
                            wp_sb[:, 4 * r + 3, 128 * cob : 128 * (cob + 1)],
                            ytB[r][:],
                            start=(r == 0), stop=(r == 3),
                        )
                    o_sb = osb_pool.tile([128, 512], BF16, tag="osb")
                    nc.vector.tensor_add(o_sb[:], ps_o[:], spills[(tch, cob)][:])
                    nc.sync.dma_start(
                        out=outT[128 * cob : 128 * (cob + 1), tcs], in_=o_sb[:]
                    )

    nc.compile()
    return nc


def _get_nc():
    if "nc" not in _CACHE:
        _CACHE["nc"] = _build()
    return _CACHE["nc"]


def _host_inputs(x, cos, sin, Wq, Wk, Wv, Wp):
    bf16 = ml_dtypes.bfloat16
    x = np.asarray(x)
    cos = np.asarray(cos, dtype=np.float32)
    sin = np.asarray(sin, dtype=np.float32)
    cosT = np.ascontiguousarray(np.concatenate([cos.T, cos.T], axis=0)).astype(bf16)
    sinT = np.ascontiguousarray(np.concatenate([sin.T, -sin.T], axis=0)).astype(bf16)
    p = np.arange(128)[:, None]
    j = np.arange(512)[None, :]
    masks = np.stack([(j >= p + 128 * d) for d in range(4)], axis=0).astype(bf16)

    in_maps = []
    for core in range(8):
        b, g = core // 4, core % 4
        in_maps.append(
            {
                "xT": np.ascontiguousarray(np.asarray(x)[b].T).astype(bf16),
                "wq": np.ascontiguousarray(
                    Wq[:, 512 * g : 512 * g + 512]
                ).astype(bf16),
                "wk": np.ascontiguousarray(
                    Wk[:, 128 * g : 128 * g + 128]
                ).astype(bf16),
                "wv": np.ascontiguousarray(
                    Wv[:, 128 * g : 128 * g + 128]
                ).astype(bf16),
                "wp": np.ascontiguousarray(
                    Wp[:, 512 * g : 512 * g + 512]
                ).astype(bf16),
                "cosT": cosT,
                "sinT": sinT,
                "masks": masks,
            }
        )
    return in_maps


def kernel(x, cos, sin, Wq, Wk, Wv, Wp):
    from concourse.bass_utils import run_bass_kernel_spmd

    in_maps = _host_inputs(x, cos, sin, Wq, Wk, Wv, Wp)
    nc = _get_nc()
    res = run_bass_kernel_spmd(nc, in_maps, core_ids=list(range(8)), trace=False)

    out = np.empty((B, T, C), dtype=np.float32)
    for core in range(8):
        b, g = core // 4, core % 4
        out[b, :, 512 * g : 512 * g + 512] = (
            res.results[core]["outT"].T.astype(np.float32)
        )
    return out



# revision 5
# speedup vs baseline: 1.0695x; 1.0695x over previous
"""Causal self-attention (GQA + RoPE + QK-RMSNorm) on 8 trn2 NeuronCores.

Reference (B=2, T=2048, C=2048, 16 q-heads / 4 kv-heads, head_dim 128):
    q = rms_norm(rope(x @ Wq)) / sqrt(128); k = rms_norm(rope(x @ Wk))
    att = softmax_causal(q k^T / sqrt(128)); y = (att @ v) @ Wp
Sharding: core = 4*b + g  (b = batch 0..1, g = head-group 0..3).
Each core computes q-heads 4g..4g+3 (kv-head g), attends over the full
causal sequence of its batch, and produces a 512-column slice of the
output projection. Host concatenates.

Fully fused single pass over 512-t chunks tc:
  1. project chunk tc -> K^T (rms-folded), V (PE-transposed), Q^T x4
  2. output-projection for chunk tc-1 (all 16 heads in one PSUM
     accumulation; own 4 heads read from SBUF, 12 remote heads from the
     AllGather output landed during the previous iteration)
  3. attention for query-chunk tc, all 4 heads, keys 0..512*(tc+1)
  4. AllGather of this chunk's 4 yT tiles (per-chunk DRAM tiles so a
     running collective never write-after-read-blocks later stores)

Row-sum of exp'd scores: pair-halves added on DVE, one rs matmul per
pair (PSUM-accumulated) instead of one per key block.  Queue discipline:
gpsimd = x prefetch + broadcasts + yT stores + AG triggers, sync = AG
output loads only (it may block on the collective harmlessly), vector =
out-tile evac + outT stores, scalar = activations only.
"""

import ml_dtypes
import numpy as np

B, T, C = 2, 2048, 2048
NH, NKV, HD = 16, 4, 128
G = 4  # q-heads per core
EPS = 1e-6
NCB = C // 128  # 16 contraction blocks
NTCH = T // 512  # 4 t-chunks

_CACHE = {}


def _build():
    import concourse.mybir as mybir
    import concourse.tile as tile
    from concourse import bacc
    from concourse.masks import make_identity
    from contextlib import ExitStack

    F32 = mybir.dt.float32
    BF16 = mybir.dt.bfloat16
    AF = mybir.ActivationFunctionType

    nc = bacc.Bacc(None, target_bir_lowering=False, num_devices=8)

    xT = nc.dram_tensor("xT", [C, T], BF16, kind="ExternalInput")
    wq = nc.dram_tensor("wq", [C, G * HD], BF16, kind="ExternalInput")
    wk = nc.dram_tensor("wk", [C, HD], BF16, kind="ExternalInput")
    wv = nc.dram_tensor("wv", [C, HD], BF16, kind="ExternalInput")
    wp = nc.dram_tensor("wp", [C, G * HD], BF16, kind="ExternalInput")
    cosT = nc.dram_tensor("cosT", [128, T], BF16, kind="ExternalInput")
    sinT = nc.dram_tensor("sinT", [128, T], BF16, kind="ExternalInput")
    masks = nc.dram_tensor("masks", [4, 128, 512], BF16, kind="ExternalInput")
    outT = nc.dram_tensor("outT", [G * HD, T], BF16, kind="ExternalOutput")

    with tile.TileContext(nc) as tc_ctx:
        with ExitStack() as S:
            dram = S.enter_context(tc_ctx.tile_pool(name="dram", bufs=1, space="DRAM"))
            ag_in = [
                dram.tile([4 * HD, 512], BF16, name=f"ag_in_{t}") for t in range(NTCH)
            ]
            ag_out = [
                dram.tile([16 * HD, 512], BF16, name=f"ag_out_{t}")
                for t in range(NTCH)
            ]

            consts = S.enter_context(tc_ctx.tile_pool(name="consts", bufs=1))
            ones_bf = consts.tile([128, 1], BF16)
            nc.vector.memset(ones_bf[:], 1.0)
            eps_k = consts.tile([1, 1], F32)
            nc.vector.memset(eps_k[:], EPS)
            eps_q = consts.tile([1, 1], F32)
            nc.vector.memset(eps_q[:], float(HD * HD) * EPS)
            ident_bf = consts.tile([128, 128], BF16)
            make_identity(nc, ident_bf[:])

            wpool = S.enter_context(tc_ctx.tile_pool(name="w", bufs=1))
            wq_sb = wpool.tile([128, NCB, G * HD], BF16)
            wk_sb = wpool.tile([128, NCB, HD], BF16)
            wv_sb = wpool.tile([128, NCB, HD], BF16)
            wp_sb = wpool.tile([128, NCB, G * HD], BF16)
            wkr = wk.rearrange("(cb p) n -> p cb n", p=128)
            wvr = wv.rearrange("(cb p) n -> p cb n", p=128)
            wqr = wq.rearrange("(cb p) n -> p cb n", p=128)
            wpr = wp.rearrange("(cb p) n -> p cb n", p=128)
            # wk first: chunk-0 K projection is the first PE work.
            for i in range(2):
                cbs = slice(8 * i, 8 * i + 8)
                nc.scalar.dma_start(out=wk_sb[:, cbs, :], in_=wkr[:, cbs, :])

            trig = S.enter_context(tc_ctx.tile_pool(name="trig", bufs=1))
            cos_sb = trig.tile([128, T], BF16)
            sin_sb = trig.tile([128, T], BF16)
            masks_sb = trig.tile([128, 4, 512], BF16)

            acts = S.enter_context(tc_ctx.tile_pool(name="acts", bufs=1))
            qT_sb = acts.tile([128, G, T], BF16)
            kT_sb = acts.tile([128, T], BF16)
            v_sb = acts.tile([128, NCB, HD], BF16)
            yT_sb = acts.tile([128, G, T], BF16)

            xt_pool = S.enter_context(tc_ctx.tile_pool(name="xt", bufs=8))
            rem_pool = S.enter_context(tc_ctx.tile_pool(name="rem", bufs=6))
            tmp = S.enter_context(tc_ctx.tile_pool(name="tmp", bufs=2))
            rowp = S.enter_context(tc_ctx.tile_pool(name="rowp", bufs=4))
            pt_pool = S.enter_context(tc_ctx.tile_pool(name="pt", bufs=3))
            pairs_pool = S.enter_context(tc_ctx.tile_pool(name="pairs", bufs=2))
            osb_pool = S.enter_context(tc_ctx.tile_pool(name="osb", bufs=3))

            # PSUM: acc 3 + sp 2x2 + rows 1 = 8 banks
            acc = S.enter_context(tc_ctx.tile_pool(name="acc", bufs=3, space="PSUM"))
            spp = S.enter_context(tc_ctx.tile_pool(name="spp", bufs=2, space="PSUM"))
            rows = S.enter_context(tc_ctx.tile_pool(name="rows", bufs=1, space="PSUM"))

            def rope_norm(dst, psrc, tcs, sqrt_scale, sqrt_bias):
                """dst = rope(psrc) / sqrt(sqrt_scale*ssq + bias), bf16 math."""
                xb = tmp.tile([128, 512], BF16, tag="xb")
                nc.scalar.copy(out=xb[:], in_=psrc)
                rot = tmp.tile([128, 512], BF16, tag="rot")
                # sin_sb rows 0-63 hold +sin, rows 64-127 hold -sin, so each
                # tensor_tensor reads both SBUF operands at the same base
                # partition (compiler constraint NCC_IBIR297).
                nc.vector.tensor_mul(rot[0:64, :], xb[64:128, :], sin_sb[64:128, tcs])
                nc.vector.tensor_mul(rot[64:128, :], xb[0:64, :], sin_sb[0:64, tcs])
                xc = tmp.tile([128, 512], BF16, tag="xc")
                nc.vector.tensor_mul(xc[:], xb[:], cos_sb[:, tcs])
                ro = tmp.tile([128, 512], BF16, tag="ro")
                nc.vector.tensor_add(ro[:], xc[:], rot[:])
                sq = tmp.tile([128, 512], BF16, tag="sq")
                nc.vector.tensor_mul(sq[:], ro[:], ro[:])
                ps_ss = rows.tile([1, 512], F32, tag="rows")
                nc.tensor.matmul(ps_ss[:], ones_bf[:], sq[:], start=True, stop=True)
                srow = rowp.tile([1, 512], F32, tag="srow")
                nc.scalar.activation(
                    out=srow[:], in_=ps_ss[:], func=AF.Sqrt,
                    scale=sqrt_scale, bias=sqrt_bias,
                )
                rrow = rowp.tile([1, 512], F32, tag="rrow")
                nc.vector.reciprocal_approx_fast(out=rrow[:], in_=srow[:])
                bc = tmp.tile([128, 512], F32, tag="bc")
                nc.gpsimd.partition_broadcast(bc[:], rrow[:])
                nc.vector.tensor_mul(dst, ro[:], bc[:])

            def load_x_chunk(tcn, first):
                """Prefetch x chunk tcn; returns the 4 SBUF tiles."""
                tcs = slice(512 * tcn, 512 * tcn + 512)
                xts = []
                for gx in range(4):
                    xt = xt_pool.tile([128, 4, 512], BF16, tag="xt")
                    xr = xT[512 * gx : 512 * (gx + 1), tcs].rearrange(
                        "(cb p) t -> p cb t", p=128
                    )
                    if first:
                        # startup-critical: fine-grained round-robin
                        engs = [nc.sync, nc.gpsimd, nc.scalar]
                        for j in range(4):
                            engs[(4 * gx + j) % 3].dma_start(
                                out=xt[:, j : j + 1, :], in_=xr[:, j : j + 1, :]
                            )
                    else:
                        eng = nc.sync if gx < 2 else nc.gpsimd
                        eng.dma_start(out=xt[:, 0:2, :], in_=xr[:, 0:2, :])
                        eng.dma_start(out=xt[:, 2:4, :], in_=xr[:, 2:4, :])
                    xts.append(xt)
                return xts

            def proj_chunk(tcn, xts):
                """Q/K/V projections + rope + rms-norm for chunk tcn."""
                tcs = slice(512 * tcn, 512 * tcn + 512)

                def xmov(cb):
                    return xts[cb // 4][:, cb % 4, :]

                ps_k = acc.tile([128, 512], F32, tag="acc")
                for cb in range(NCB):
                    nc.tensor.matmul(
                        ps_k[:], wk_sb[:, cb, :], xmov(cb),
                        start=(cb == 0), stop=(cb == NCB - 1),
                    )
                rope_norm(kT_sb[:, tcs], ps_k[:], tcs, 1.0 / HD, eps_k[:])

                ps_v = acc.tile([128, 512], F32, tag="acc")
                for cb in range(NCB):
                    nc.tensor.matmul(
                        ps_v[:], wv_sb[:, cb, :], xmov(cb),
                        start=(cb == 0), stop=(cb == NCB - 1),
                    )
                vb = tmp.tile([128, 512], BF16, tag="vb")
                nc.scalar.copy(out=vb[:], in_=ps_v[:])
                ps_tr = acc.tile([128, 512], BF16, tag="acc")
                for tt in range(4):
                    nc.tensor.transpose(
                        ps_tr[:, 128 * tt : 128 * (tt + 1)],
                        vb[:, 128 * tt : 128 * (tt + 1)],
                        ident_bf[:],
                    )
                for tt in range(4):
                    nc.vector.tensor_copy(
                        out=v_sb[:, 4 * tcn + tt, :],
                        in_=ps_tr[:, 128 * tt : 128 * (tt + 1)],
                    )

                for hq in range(G):
                    ps_q = acc.tile([128, 512], F32, tag="acc")
                    for cb in range(NCB):
                        nc.tensor.matmul(
                            ps_q[:],
                            wq_sb[:, cb, 128 * hq : 128 * (hq + 1)],
                            xmov(cb),
                            start=(cb == 0), stop=(cb == NCB - 1),
                        )
                    rope_norm(qT_sb[:, hq, tcs], ps_q[:], tcs, float(HD), eps_q[:])

            def attention_chunk(tcn):
                """Attention for query chunk tcn, all G heads; stores yT
                slices to ag_in[tcn] and fires the AllGather."""
                tqs = slice(512 * tcn, 512 * tcn + 512)
                nblk = 4 * (tcn + 1)
                for hq in range(G):
                    ps_y = acc.tile([128, 512], F32, tag="acc")
                    rs = rows.tile([1, 512], F32, tag="rows")
                    npr = nblk // 2
                    for pr in range(npr):
                        sp = spp.tile([128, 1024], F32, tag="sp")
                        for h2 in range(2):
                            tkb = 2 * pr + h2
                            nc.tensor.matmul(
                                sp[:, 512 * h2 : 512 * (h2 + 1)],
                                kT_sb[:, 128 * tkb : 128 * (tkb + 1)],
                                qT_sb[:, hq, tqs],
                                start=True, stop=True,
                            )
                        pT = pt_pool.tile([128, 1024], BF16, tag="pt")
                        nc.scalar.activation(out=pT[:], in_=sp[:], func=AF.Exp)
                        for h2 in range(2):
                            tkb = 2 * pr + h2
                            d = tkb - 4 * tcn
                            if d >= 0:
                                hs = slice(512 * h2, 512 * (h2 + 1))
                                nc.vector.tensor_mul(
                                    pT[:, hs], pT[:, hs], masks_sb[:, d, :]
                                )
                        # pair-sum on DVE -> one rs matmul per pair
                        pms = pairs_pool.tile([128, 512], BF16, tag="pms")
                        nc.vector.tensor_add(
                            pms[:], pT[:, 0:512], pT[:, 512:1024]
                        )
                        nc.tensor.matmul(
                            rs[:], ones_bf[:], pms[:],
                            start=(pr == 0), stop=(pr == npr - 1),
                        )
                        for h2 in range(2):
                            tkb = 2 * pr + h2
                            hs = slice(512 * h2, 512 * (h2 + 1))
                            nc.tensor.matmul(
                                ps_y[:], v_sb[:, tkb, :], pT[:, hs],
                                start=(tkb == 0), stop=(tkb == nblk - 1),
                            )
                    rrow = rowp.tile([1, 512], F32, tag="rrow2")
                    nc.vector.reciprocal_approx_fast(out=rrow[:], in_=rs[:])
                    bc = tmp.tile([128, 512], F32, tag="bc2")
                    nc.gpsimd.partition_broadcast(bc[:], rrow[:])
                    yt = yT_sb[:, hq, tqs]
                    nc.vector.tensor_mul(yt, ps_y[:], bc[:])
                    nc.gpsimd.dma_start(
                        out=ag_in[tcn][128 * hq : 128 * (hq + 1), :], in_=yt
                    )
                nc.gpsimd.collective_compute(
                    "AllGather",
                    mybir.AluOpType.bypass,
                    replica_groups=[[0, 1, 2, 3], [4, 5, 6, 7]],
                    ins=[ag_in[tcn][:]],
                    outs=[ag_out[tcn][:]],
                )

            def proj_out(tcn, my_g_dummy):
                """Output projection for chunk tcn using all 16 heads."""
                tcs = slice(512 * tcn, 512 * tcn + 512)
                # load the 12 remote head tiles (3 contiguous 512-row
                # segments of ag_out); own 4 heads come from yT_sb.
                rem = {}
                for s in range(4):
                    rt = rem_pool.tile([128, 4, 512], BF16, tag="rem", name=f"rem{tcn}_{s}")
                    nc.sync.dma_start(
                        out=rt[:],
                        in_=ag_out[tcn][512 * s : 512 * (s + 1), :].rearrange(
                            "(b p) t -> p b t", p=128
                        ),
                    )
                    rem[s] = rt
                for cob in range(4):
                    ps_o = acc.tile([128, 512], F32, tag="acc")
                    for r in range(NCB):
                        s, j = r // 4, r % 4
                        rhs = rem[s][:, j, :]
                        nc.tensor.matmul(
                            ps_o[:],
                            wp_sb[:, r, 128 * cob : 128 * (cob + 1)],
                            rhs,
                            start=(r == 0), stop=(r == NCB - 1),
                        )
                    o_sb = osb_pool.tile([128, 512], BF16, tag="osb")
                    nc.vector.tensor_copy(out=o_sb[:], in_=ps_o[:])
                    nc.sync.dma_start(
                        out=outT[128 * cob : 128 * (cob + 1), tcs], in_=o_sb[:]
                    )

            # ---- main fused loop ----
            xts_cur = load_x_chunk(0, first=True)
            # remaining startup loads, ordered by first use
            nc.scalar.dma_start(out=cos_sb[:], in_=cosT[:])
            nc.scalar.dma_start(out=sin_sb[:], in_=sinT[:])
            for i in range(2):
                cbs = slice(8 * i, 8 * i + 8)
                nc.scalar.dma_start(out=wv_sb[:, cbs, :], in_=wvr[:, cbs, :])
            for i in range(4):
                cbs = slice(4 * i, 4 * i + 4)
                eng = nc.sync if i % 2 == 0 else nc.scalar
                eng.dma_start(out=wq_sb[:, cbs, :], in_=wqr[:, cbs, :])
            nc.sync.dma_start(out=masks_sb[:], in_=masks.rearrange("d p m -> p d m"))

            for tcn in range(NTCH):
                if tcn + 1 < NTCH:
                    xts_next = load_x_chunk(tcn + 1, first=False)
                if tcn == 0:
                    # wp needed first at proj_out(0), during iteration 1
                    for i in range(4):
                        cbs = slice(4 * i, 4 * i + 4)
                        nc.sync.dma_start(out=wp_sb[:, cbs, :], in_=wpr[:, cbs, :])
                proj_chunk(tcn, xts_cur)
                attention_chunk(tcn)
                if tcn > 0:
                    proj_out(tcn - 1, None)
                if tcn + 1 < NTCH:
                    xts_cur = xts_next
            proj_out(NTCH - 1, None)

    nc.compile()
    return nc


def _get_nc():
    if "nc" not in _CACHE:
        _CACHE["nc"] = _build()
    return _CACHE["nc"]


def _host_inputs(x, cos, sin, Wq, Wk, Wv, Wp):
    bf16 = ml_dtypes.bfloat16
    x = np.asarray(x)
    cos = np.asarray(cos, dtype=np.float32)
    sin = np.asarray(sin, dtype=np.float32)
    cosT = np.ascontiguousarray(np.concatenate([cos.T, cos.T], axis=0)).astype(bf16)
    sinT = np.ascontiguousarray(np.concatenate([sin.T, -sin.T], axis=0)).astype(bf16)
    p = np.arange(128)[:, None]
    j = np.arange(512)[None, :]
    masks = np.stack([(j >= p + 128 * d) for d in range(4)], axis=0).astype(bf16)

    in_maps = []
    for core in range(8):
        b, g = core // 4, core % 4
        in_maps.append(
            {
                "xT": np.ascontiguousarray(np.asarray(x)[b].T).astype(bf16),
                "wq": np.ascontiguousarray(
                    Wq[:, 512 * g : 512 * g + 512]
                ).astype(bf16),
                "wk": np.ascontiguousarray(
                    Wk[:, 128 * g : 128 * g + 128]
                ).astype(bf16),
                "wv": np.ascontiguousarray(
                    Wv[:, 128 * g : 128 * g + 128]
                ).astype(bf16),
                "wp": np.ascontiguousarray(
                    Wp[:, 512 * g : 512 * g + 512]
                ).astype(bf16),
                "cosT": cosT,
                "sinT": sinT,
                "masks": masks,
            }
        )
    return in_maps


def kernel(x, cos, sin, Wq, Wk, Wv, Wp):
    from concourse.bass_utils import run_bass_kernel_spmd

    in_maps = _host_inputs(x, cos, sin, Wq, Wk, Wv, Wp)
    nc = _get_nc()
    res = run_bass_kernel_spmd(nc, in_maps, core_ids=list(range(8)), trace=False)

    out = np.empty((B, T, C), dtype=np.float32)
    for core in range(8):
        b, g = core // 4, core % 4
        out[b, :, 512 * g : 512 * g + 512] = (
            res.results[core]["outT"].T.astype(np.float32)
        )
    return out
